# revision 21
# baseline (speedup 1.0000x reference)
"""CWCT (class-wise whitening/coloring transform) for Trainium2, 8 NeuronCores.

Strategy
--------
Pixels are counting-sorted by segment label on the host (pure data
movement); each label's pixel range is split contiguously across the 8
cores, zero-padded to a fixed per-(core,label) capacity.

Device phase 1 (per core): for every label, accumulate the raw second
moment S_l = sum_p x_p x_p^T over that core's pixel shard, for content
and style, as fp8(e4m3) DoubleRow matmuls (256-pixel contraction per
instruction, f32 PSUM accumulate) — fp8 quantization noise averages out
over the ~32k-pixel sums and halves both HBM traffic and PE cycles vs
bf16. Dual-row fp8 LDWEIGHTS requires its two 128-row sub-blocks exactly
256 bytes apart, so the row is exactly the 256 channels — the channel
sums (for the means) are instead computed exactly on the host in f64
from the already-sorted pixel lists.

Host middle: all-reduce the (tiny) per-core partial moments, form
covariances, Cholesky factors, inv_Lc via triangular solve (float64),
combined transform T_l = Ls @ inv_Lc and bias b_l = mu_s - T_l mu_c.
Invalid labels get T = I, b = 0 (and are restored exactly from the
original content on the host at assembly time).

Device phase 2 (per core): colored = T_l @ x + b_l applied per label in
bf16 (output precision) with T stationary in the PE array, streaming
channel-major pixel blocks.

Host end: scatter the colored pixels back to the original pixel order.
"""

import numpy as np
import ml_dtypes

import concourse.bacc as bacc
import concourse.mybir as mybir
import concourse.tile as tile
from concourse.bass_utils import run_bass_kernel_spmd

NCORES = 8
BF16 = ml_dtypes.bfloat16
FP8 = ml_dtypes.float8_e4m3
ROW = 256  # dual-row fp8 LDWEIGHTS: the two 128-row sub-blocks must be 256B apart

# set by test harness to capture profiles
TRACE = False
TRACE_DIR = "/tmp/cwct_trace"
LAST_NS = {}
# overlap phase-2's NEFF compile (background thread + dummy run) with phase 1
PRECOMPILE_WARM = True


def _round_up(x, m):
    return (int(x) + m - 1) // m * m


P1_KT = 40  # 256-px tiles per phase-1 DMA group -> 20KB per-partition rows


def _p1_groups(total_tiles, ramp=False):
    """Label-agnostic DMA group tile counts covering a whole feature's
    tile stream: uniform big groups (large contiguous per-partition DMA
    rows for full HBM bandwidth), with a small trailing group so the
    end-of-stream compute+evict tail is short. With ramp=True (first
    feature only) the stream opens with small groups so the tensor
    engine starts within ~1.5us of the first DMA instead of waiting for
    a full 16KB-row group."""
    kts = []
    rem = total_tiles
    if ramp:
        for r in (4, 8, 16):
            if rem <= r + 4:
                break
            kts.append(r)
            rem -= r
    while rem > P1_KT:
        kts.append(P1_KT)
        rem -= P1_KT
    if rem > 4:
        kts += [rem - 4, 4]
    elif rem:
        kts.append(rem)
    return kts


def _p2_groups(C2s):
    """Per-label pixel-group splits: chunks of <=1024 px per DMA, the
    very last group of the last label trimmed to 128 px so the final
    DMA->matmul->evict->DMA chain is short. Each chunk also carries its
    <=512-px PSUM sub-blocks."""
    L = len(C2s)
    out = []
    for l in range(L):
        rem = C2s[l]
        chunks = []
        while rem > 0:
            g = min(2048, rem)
            chunks.append(g)
            rem -= g
        if l == L - 1 and chunks and chunks[-1] > 128:
            chunks = chunks[:-1] + [chunks[-1] - 128, 128]
        off = 0
        groups = []
        for g in chunks:
            subs = []
            so = 0
            while so < g:
                s = min(512, g - so)
                subs.append((so, s))
                so += s
            groups.append((off, g, subs))
            off += g
        out.append(groups)
    return out


def _build_phase1(T2c, T2s, N):
    """Inputs gc/gs: flat fp8 host-swizzled arrays, chunked into
    label-agnostic (128, KT, 2, ROW) DMA groups (16KB contiguous
    per-partition rows). Per 256-px tile k the two DoubleRow matmuls
    contract both 128-row sub-blocks at once.
    Outputs sc/ss: (L, 128, 384) f16 per label row block (f16 halves the
    output-DMA fabric time; partial-sum quantization is ~1e-4 of cov):
    [:, 0:256]   = S[0:128, 0:256] (upper row block, all columns)
    [:, 256:384] = S[128:256, 128:256] (lower-right block)
    (S[128:256, 0:128] is recovered on the host as S[0:128,128:256].T)"""
    assert N == 256
    L = len(T2c)
    W = 2 * N - 128  # 384
    DR = mybir.MatmulPerfMode.DoubleRow
    tot_c = sum(T2c) * 128 * 2 * ROW
    tot_s = sum(T2s) * 128 * 2 * ROW
    nc = bacc.Bacc("TRN2", target_bir_lowering=False, debug=False, num_devices=NCORES)
    gc = nc.dram_tensor("gc", [tot_c], mybir.dt.float8e4, kind="ExternalInput")
    gs = nc.dram_tensor("gs", [tot_s], mybir.dt.float8e4, kind="ExternalInput")
    sc = nc.dram_tensor("sc", [L, 128, W], mybir.dt.float16, kind="ExternalOutput")
    ss = nc.dram_tensor("ss", [L, 128, W], mybir.dt.float16, kind="ExternalOutput")

    with tile.TileContext(nc) as tc:
        with (
            tc.tile_pool(name="gin", bufs=5) as gin,
            tc.tile_pool(name="out", bufs=4) as outp,
            tc.tile_pool(name="ps", bufs=8, space="PSUM") as psum,
        ):
            for fi, (g_dram, o_dram, T2l) in enumerate(
                ((gc, sc, T2c), (gs, ss, T2s))
            ):
                # label-agnostic DMA groups over the whole tile stream
                kts = _p1_groups(sum(T2l), ramp=(fi == 0))
                bounds = []
                acc = 0
                for kt in kts:
                    bounds.append((acc, kt))
                    acc += kt
                tile_grp = []  # global tile idx -> (group idx, idx in group)
                for gidx, (g0, kt) in enumerate(bounds):
                    for i in range(kt):
                        tile_grp.append((gidx, i))
                gtiles = [None] * len(kts)
                off = 0
                gi = 0
                for l in range(L):
                    T2 = T2l[l]
                    if T2 == 0:
                        continue
                    ps0 = psum.tile([128, N], mybir.dt.float32, tag="ps")
                    ps1 = psum.tile([128, 128], mybir.dt.float32, tag="ps")
                    for n in range(T2):
                        gidx, k = tile_grp[gi]
                        if k == 0:
                            KT = kts[gidx]
                            t = gin.tile(
                                [128, P1_KT, 2, ROW], mybir.dt.float8e4, tag="g"
                            )
                            src = g_dram[off : off + 128 * KT * 2 * ROW].rearrange(
                                "(p t j c) -> p t j c", p=128, t=KT, j=2, c=ROW
                            )
                            nc.sync.dma_start(t[:, 0:KT], src)
                            off += 128 * KT * 2 * ROW
                            gtiles[gidx] = t
                        t = gtiles[gidx]
                        nc.tensor.matmul(
                            ps0[:], t[:, k, :, 0:128], t[:, k, :, :],
                            start=(n == 0), stop=(n == T2 - 1), perf_mode=DR,
                        )
                        nc.tensor.matmul(
                            ps1[:], t[:, k, :, 128:256], t[:, k, :, 128:256],
                            start=(n == 0), stop=(n == T2 - 1), perf_mode=DR,
                        )
                        gi += 1
                    ob = outp.tile([128, W], mybir.dt.float16, tag="o")
                    nc.vector.tensor_copy(ob[:, 0:N], ps0[:])
                    nc.vector.tensor_copy(ob[:, N:W], ps1[:])
                    # scalar HWDGE ring: keep the sync ring free for inputs
                    nc.scalar.dma_start(o_dram[l], ob[:])
    nc.compile()
    return nc


def _build_phase2(C2s, N):
    """g2: (N, P2) bf16 channel-major gathered content (per-label blocks
    of C2s[l] pixels).
    tq: (128, L, 2, 2, 128) bf16 with tq[k,l,j,i,m] = T_l[i*128+m, j*128+k].
    bi: (128, 2, L) f32 with bi[p,i,l] = b_l[i*128+p].
    oc: (N, P2) bf16 colored output (channel-major, gathered order)."""
    assert N == 256
    L = len(C2s)
    P2 = sum(C2s)
    lab_off = np.concatenate(([0], np.cumsum(C2s))).astype(int)

    nc = bacc.Bacc("TRN2", target_bir_lowering=False, debug=False, num_devices=NCORES)
    g2 = nc.dram_tensor("g2", [N, P2], mybir.dt.bfloat16, kind="ExternalInput")
    tq = nc.dram_tensor("tq", [128, L, 2, 2, 128], mybir.dt.bfloat16, kind="ExternalInput")
    bi = nc.dram_tensor("bi", [128, 2, L], mybir.dt.float32, kind="ExternalInput")
    oc = nc.dram_tensor("oc", [N, P2], mybir.dt.bfloat16, kind="ExternalOutput")

    groups_by_label = _p2_groups(C2s)

    with tile.TileContext(nc) as tc:
        with (
            tc.tile_pool(name="const", bufs=1) as constp,
            tc.tile_pool(name="gin", bufs=8) as gin,
            tc.tile_pool(name="out", bufs=8) as outp,
            tc.tile_pool(name="ps", bufs=4, space="PSUM") as psum,
        ):
            # constants on the scalar ring so the first pixel-block DMA is
            # not queued behind them on the sync ring
            tqt = constp.tile([128, L, 2, 2, 128], mybir.dt.bfloat16)
            nc.scalar.dma_start(tqt[:], tq[:])
            bit = constp.tile([128, 2, L], mybir.dt.float32)
            nc.scalar.dma_start(bit[:], bi[:])

            g2r = g2[:].rearrange("(j k) x -> k j x", j=2)
            ocr2 = oc[:].rearrange("(i k) x -> k i x", i=2)
            for l in range(L):
                base = int(lab_off[l])
                for off, G, subs in groups_by_label[l]:
                    gt = gin.tile([128, 2, 2048], mybir.dt.bfloat16, tag="g")
                    nc.sync.dma_start(
                        gt[:, :, 0:G], g2r[:, :, base + off : base + off + G]
                    )
                    # both i-chunks evict into one tile -> a single output
                    # DMA per group
                    ob = outp.tile([128, 2, 2048], mybir.dt.bfloat16, tag="o")
                    for i in range(2):
                        for so, S in subs:
                            # one PSUM bank per <=512-px sub-block
                            ps = psum.tile([128, 512], mybir.dt.float32, tag="ps")
                            nc.tensor.matmul(
                                ps[:, 0:S], tqt[:, l, 0, i, :],
                                gt[:, 0, so : so + S], start=True, stop=False,
                            )
                            nc.tensor.matmul(
                                ps[:, 0:S], tqt[:, l, 1, i, :],
                                gt[:, 1, so : so + S], start=False, stop=True,
                            )
                            # evictions split across the two elementwise
                            # engines so neither stalls PSUM recycling
                            if i == 0:
                                nc.vector.tensor_scalar_add(
                                    ob[:, 0, so : so + S], ps[:, 0:S],
                                    bit[:, i, l : l + 1],
                                )
                            else:
                                nc.scalar.activation(
                                    ob[:, 1, so : so + S], ps[:, 0:S],
                                    mybir.ActivationFunctionType.Identity,
                                    bias=bit[:, i, l : l + 1],
                                )
                    # gpsimd HWDGE ring: keeps the scalar engine free for
                    # evictions and the sync ring free for inputs
                    nc.gpsimd.dma_start(
                        ocr2[:, :, base + off : base + off + G], ob[:, :, 0:G]
                    )
    nc.compile()
    return nc


def _run(nc, in_maps, label):
    if TRACE:
        import os
        import shutil

        tdir = f"{TRACE_DIR}/{label}"
        shutil.rmtree(tdir, ignore_errors=True)
        os.makedirs(tdir, exist_ok=True)
        res = run_bass_kernel_spmd(
            nc, in_maps, list(range(NCORES)), trace=True, tmpdir=tdir
        )
        LAST_NS[label] = res.exec_time_ns
    else:
        res = run_bass_kernel_spmd(nc, in_maps, list(range(NCORES)))
    return res


def kernel(content_feat, style_feat, content_seg, style_seg, num_labels):
    L = int(num_labels)
    B, N, H, W = content_feat.shape
    M = H * W
    assert B == 1 and N == 256

    c = np.asarray(content_feat, dtype=np.float32).reshape(N, M)
    s = np.asarray(style_feat, dtype=np.float32).reshape(N, M)
    seg_c = np.asarray(content_seg).reshape(M).astype(np.int64)
    seg_s = np.asarray(style_seg).reshape(M).astype(np.int64)

    order_c = np.argsort(seg_c, kind="stable")
    order_s = np.argsort(seg_s, kind="stable")
    counts_c = np.bincount(seg_c, minlength=L)[:L]
    counts_s = np.bincount(seg_s, minlength=L)[:L]

    def split_counts(cnt):
        base = cnt // NCORES
        out = np.tile(base[:, None], (1, NCORES))
        for l in range(L):
            out[l, : cnt[l] % NCORES] += 1
        return out

    cc = split_counts(counts_c)  # (L, NCORES)
    cs = split_counts(counts_s)

    # phase-1 per-label 256-px tile counts; phase-2 per-label 128-px caps
    T2c = [int(_round_up(cc[l].max(), 256)) // 256 for l in range(L)]
    T2s = [int(_round_up(cs[l].max(), 256)) // 256 for l in range(L)]
    C2s = [int(_round_up(cc[l].max(), 128)) for l in range(L)]
    lab_off2 = np.concatenate(([0], np.cumsum(C2s))).astype(int)
    P2 = int(lab_off2[-1])

    # sort the pixel-major views once; everything else slices contiguously
    cT32 = np.ascontiguousarray(c.T)  # (M, N)
    sorted_c = cT32[order_c]  # (M, N) label-sorted pixel-major content
    sorted_s = np.ascontiguousarray(s.T)[order_s]
    lab_pos_c = np.concatenate(([0], np.cumsum(counts_c))).astype(int)
    lab_pos_s = np.concatenate(([0], np.cumsum(counts_s))).astype(int)

    # exact (f64) per-label channel sums on the host; the device computes
    # only the raw second moments
    sum_c_host = np.zeros((L, N), dtype=np.float64)
    sum_s_host = np.zeros((L, N), dtype=np.float64)
    for l in range(L):
        sum_c_host[l] = sorted_c[lab_pos_c[l] : lab_pos_c[l + 1]].sum(
            axis=0, dtype=np.float64
        )
        sum_s_host[l] = sorted_s[lab_pos_s[l] : lab_pos_s[l + 1]].sum(
            axis=0, dtype=np.float64
        )

    sorted_c_f8 = sorted_c.astype(FP8)
    sorted_s_f8 = sorted_s.astype(FP8)

    def build_gather_p1(sx_f8, core_counts, lab_pos, T2l, ramp):
        """Per core, a flat fp8 array: the whole feature's 256-px tile
        stream (per-label zero-padded blocks back to back), chunked into
        label-agnostic _p1_groups DMA groups, each laid out
        (128, KT, 2, ROW); pixel t*256+j*128+p of the stream lands at
        [p, t, j, :] of its group."""
        ntiles = sum(T2l)
        kts = _p1_groups(ntiles, ramp=ramp)
        arrs = []
        for k in range(NCORES):
            tiles = np.zeros((ntiles * 256, ROW), dtype=FP8)
            t0 = 0
            for l in range(len(T2l)):
                T2 = T2l[l]
                if T2 == 0:
                    continue
                m = int(core_counts[l, k])
                off = int(lab_pos[l]) + sum(int(core_counts[l, kk]) for kk in range(k))
                if m:
                    tiles[t0 * 256 : t0 * 256 + m] = sx_f8[off : off + m]
                t0 += T2
            tiles = tiles.reshape(ntiles, 2, 128, ROW)
            out = np.empty(ntiles * 128 * 2 * ROW, dtype=FP8)
            pos = 0
            g0 = 0
            for kt in kts:
                nel = kt * 128 * 2 * ROW
                out[pos : pos + nel] = (
                    tiles[g0 : g0 + kt].transpose(2, 0, 1, 3).reshape(-1)
                )
                pos += nel
                g0 += kt
            arrs.append(out)
        return arrs

    gc_arrs = build_gather_p1(sorted_c_f8, cc, lab_pos_c, T2c, ramp=True)
    gs_arrs = build_gather_p1(sorted_s_f8, cs, lab_pos_s, T2s, ramp=False)
    del sorted_c_f8, sorted_s_f8, sorted_s

    # kick off phase-2 build + a dummy warm-up run in the background so its
    # NEFF compile overlaps phase 1's (wall-clock only; device results of the
    # dummy run are discarded). Falls back to the serial path on any failure.
    p2_box = {}

    def _precompile_p2():
        try:
            nc2 = _build_phase2(C2s, N)
            if PRECOMPILE_WARM:
                z = {
                    "g2": np.zeros((N, P2), dtype=BF16),
                    "tq": np.zeros((128, L, 2, 2, 128), dtype=BF16),
                    "bi": np.zeros((128, 2, L), dtype=np.float32),
                }
                run_bass_kernel_spmd(nc2, [z] * NCORES, list(range(NCORES)))
            p2_box["nc"] = nc2
        except Exception as e:  # pragma: no cover - fallback path
            p2_box["err"] = e

    import threading

    p2_thread = threading.Thread(target=_precompile_p2, daemon=True)
    p2_thread.start()

    nc1p = _build_phase1(T2c, T2s, N)
    if TRACE:
        # keep the traced phase-1 profile free of the background warm-up run
        p2_thread.join()
    res1 = _run(
        nc1p,
        [{"gc": gc_arrs[k], "gs": gs_arrs[k]} for k in range(NCORES)],
        "p1",
    )

    # host: all-reduce moments, finish stats, cholesky, transforms (float64)
    PW = 2 * N - 128
    sc_sum = np.zeros((L, 128, PW), dtype=np.float64)
    ss_sum = np.zeros((L, 128, PW), dtype=np.float64)
    for k in range(NCORES):
        sc_sum += res1.results[k]["sc"]
        ss_sum += res1.results[k]["ss"]

    def unpack(ssum, l):
        Sm = np.empty((N, N), dtype=np.float64)
        Sm[0:128, :] = ssum[l, :, 0:N]
        Sm[128:N, 128:N] = ssum[l, :, N:PW]
        Sm[128:N, 0:128] = Sm[0:128, 128:N].T
        return Sm

    eyeN = np.eye(N, dtype=np.float64)
    T_all = np.zeros((L, N, N), dtype=np.float64)
    b_all = np.zeros((L, N), dtype=np.float64)
    valid = np.zeros(L, dtype=bool)

    try:
        from scipy.linalg import solve_triangular as _st

        def tri_inv(Lm):
            return _st(Lm, eyeN, lower=True)
    except ImportError:

        def tri_inv(Lm):
            return np.linalg.solve(Lm, eyeN)

    for l in range(L):
        ncnt = float(counts_c[l])
        nsnt = float(counts_s[l])
        v = (ncnt > 10) and (nsnt > 10) and (ncnt < 100.0 * nsnt) and (nsnt < 100.0 * ncnt)
        Tl, bl = eyeN, np.zeros(N)
        if v:
            Sc = unpack(sc_sum, l)
            Ss = unpack(ss_sum, l)
            mc = sum_c_host[l] / max(ncnt, 1.0)
            ms = sum_s_host[l] / max(nsnt, 1.0)
            cov_c = (Sc - ncnt * np.outer(mc, mc)) / max(max(ncnt, 1.0) - 1.0, 1.0)
            cov_s = (Ss - nsnt * np.outer(ms, ms)) / max(max(nsnt, 1.0) - 1.0, 1.0)
            try:
                Lc = np.linalg.cholesky(cov_c)
                Ls = np.linalg.cholesky(cov_s)
                Tl = Ls @ tri_inv(Lc)
                bl = ms - Tl @ mc
            except np.linalg.LinAlgError:
                v, Tl, bl = False, eyeN, np.zeros(N)
        T_all[l], b_all[l], valid[l] = Tl, bl, v

    # phase-2 inputs
    tq_np = np.zeros((128, L, 2, 2, 128), dtype=BF16)
    for l in range(L):
        Tl = T_all[l].astype(np.float32)
        for j in range(2):
            for i in range(2):
                tq_np[:, l, j, i, :] = Tl[
                    i * 128 : (i + 1) * 128, j * 128 : (j + 1) * 128
                ].T
    bi_np = np.zeros((128, 2, L), dtype=np.float32)
    for l in range(L):
        for i in range(2):
            bi_np[:, i, l] = b_all[l][i * 128 : (i + 1) * 128]

    g2_arrs = []
    for k in range(NCORES):
        g2 = np.zeros((N, P2), dtype=BF16)
        for l in range(L):
            m = int(cc[l, k])
            if m:
                off = int(lab_pos_c[l]) + sum(int(cc[l, kk]) for kk in range(k))
                dst = int(lab_off2[l])
                g2[:, dst : dst + m] = sorted_c[off : off + m].astype(BF16).T
        g2_arrs.append(g2)

    p2_thread.join()
    nc2p = p2_box.get("nc")
    if nc2p is None:
        nc2p = _build_phase2(C2s, N)
    res2 = _run(
        nc2p,
        [{"g2": g2_arrs[k], "tq": tq_np, "bi": bi_np} for k in range(NCORES)],
        "p2",
    )

    # assemble: gathered order -> sorted order -> original pixel order
    sorted_pm = np.empty((M, N), dtype=np.float32)
    pos = 0
    for l in range(L):
        for k in range(NCORES):
            m = int(cc[l, k])
            if m:
                if valid[l]:
                    dst = int(lab_off2[l])
                    sorted_pm[pos : pos + m] = np.asarray(
                        res2.results[k]["oc"].T[dst : dst + m], dtype=np.float32
                    )
                else:
                    sorted_pm[pos : pos + m] = sorted_c[pos : pos + m]
            pos += m

    # pixels whose label is outside [0, L) are untouched by the reference
    if pos < M:
        sorted_pm[pos:] = sorted_c[pos:]

    final_pm = np.empty((M, N), dtype=np.float32)
    final_pm[order_c] = sorted_pm
    return np.ascontiguousarray(final_pm.T).reshape(B, N, H, W)


# revision 22
# speedup vs baseline: 1.0103x; 1.0103x over previous
"""CWCT (class-wise whitening/coloring transform) for Trainium2, 8 NeuronCores.

Strategy
--------
Pixels are counting-sorted by segment label on the host (pure data
movement); each label's pixel range is split contiguously across the 8
cores, zero-padded to a fixed per-(core,label) capacity.

Device phase 1 (per core): for every label, accumulate the raw second
moment S_l = sum_p x_p x_p^T over that core's pixel shard, for content
and style, as fp8(e4m3) DoubleRow matmuls (256-pixel contraction per
instruction, f32 PSUM accumulate) — fp8 quantization noise averages out
over the ~32k-pixel sums and halves both HBM traffic and PE cycles vs
bf16. Dual-row fp8 LDWEIGHTS requires its two 128-row sub-blocks exactly
256 bytes apart, so the row is exactly the 256 channels — the channel
sums (for the means) are instead computed exactly on the host in f64
from the already-sorted pixel lists.

Host middle: all-reduce the (tiny) per-core partial moments, form
covariances, Cholesky factors, inv_Lc via triangular solve (float64),
combined transform T_l = Ls @ inv_Lc and bias b_l = mu_s - T_l mu_c.
Invalid labels get T = I, b = 0 (and are restored exactly from the
original content on the host at assembly time).

Device phase 2 (per core): colored = T_l @ x + b_l applied per label in
bf16 (output precision) with T stationary in the PE array, streaming
channel-major pixel blocks.

Host end: scatter the colored pixels back to the original pixel order.
"""

import numpy as np
import ml_dtypes

import concourse.bacc as bacc
import concourse.mybir as mybir
import concourse.tile as tile
from concourse.bass_utils import run_bass_kernel_spmd

NCORES = 8
BF16 = ml_dtypes.bfloat16
FP8 = ml_dtypes.float8_e4m3
ROW = 256  # dual-row fp8 LDWEIGHTS: the two 128-row sub-blocks must be 256B apart

# set by test harness to capture profiles
TRACE = False
TRACE_DIR = "/tmp/cwct_trace"
LAST_NS = {}
# overlap phase-2's NEFF compile (background thread + dummy run) with phase 1
PRECOMPILE_WARM = True


def _round_up(x, m):
    return (int(x) + m - 1) // m * m


P1_KT = 32  # 256-px tiles per phase-1 DMA group -> 16KB per-partition rows


def _p1_groups(total_tiles, ramp=False):
    """Label-agnostic DMA group tile counts covering a whole feature's
    tile stream: uniform big groups (large contiguous per-partition DMA
    rows for full HBM bandwidth), with a small trailing group so the
    end-of-stream compute+evict tail is short. With ramp=True (first
    feature only) the stream opens with small groups so the tensor
    engine starts within ~1.5us of the first DMA instead of waiting for
    a full 16KB-row group."""
    kts = []
    rem = total_tiles
    if ramp:
        for r in (4, 8, 16):
            if rem <= r + 4:
                break
            kts.append(r)
            rem -= r
    while rem > P1_KT:
        kts.append(P1_KT)
        rem -= P1_KT
    if rem > 4:
        kts += [rem - 4, 4]
    elif rem:
        kts.append(rem)
    return kts


def _p2_groups(C2s):
    """Per-label pixel-group splits: chunks of <=1024 px per DMA, the
    very last group of the last label trimmed to 128 px so the final
    DMA->matmul->evict->DMA chain is short. Each chunk also carries its
    <=512-px PSUM sub-blocks."""
    L = len(C2s)
    out = []
    for l in range(L):
        rem = C2s[l]
        chunks = []
        while rem > 0:
            g = min(1024, rem)
            chunks.append(g)
            rem -= g
        if l == L - 1 and chunks and chunks[-1] > 128:
            chunks = chunks[:-1] + [chunks[-1] - 128, 128]
        off = 0
        groups = []
        for g in chunks:
            subs = []
            so = 0
            while so < g:
                s = min(512, g - so)
                subs.append((so, s))
                so += s
            groups.append((off, g, subs))
            off += g
        out.append(groups)
    return out


def _build_phase1(T2c, T2s, N):
    """Inputs gc/gs: flat fp8 host-swizzled arrays, chunked into
    label-agnostic (128, KT, 2, ROW) DMA groups (16KB contiguous
    per-partition rows). Per 256-px tile k the two DoubleRow matmuls
    contract both 128-row sub-blocks at once.
    Outputs sc/ss: (L, 128, 384) f16 per label row block (f16 halves the
    output-DMA fabric time; partial-sum quantization is ~1e-4 of cov):
    [:, 0:256]   = S[0:128, 0:256] (upper row block, all columns)
    [:, 256:384] = S[128:256, 128:256] (lower-right block)
    (S[128:256, 0:128] is recovered on the host as S[0:128,128:256].T)"""
    assert N == 256
    L = len(T2c)
    W = 2 * N - 128  # 384
    DR = mybir.MatmulPerfMode.DoubleRow
    tot_c = sum(T2c) * 128 * 2 * ROW
    tot_s = sum(T2s) * 128 * 2 * ROW
    nc = bacc.Bacc("TRN2", target_bir_lowering=False, debug=False, num_devices=NCORES)
    gc = nc.dram_tensor("gc", [tot_c], mybir.dt.float8e4, kind="ExternalInput")
    gs = nc.dram_tensor("gs", [tot_s], mybir.dt.float8e4, kind="ExternalInput")
    sc = nc.dram_tensor("sc", [L, 128, W], mybir.dt.float16, kind="ExternalOutput")
    ss = nc.dram_tensor("ss", [L, 128, W], mybir.dt.float16, kind="ExternalOutput")

    with tile.TileContext(nc) as tc:
        with (
            tc.tile_pool(name="gin", bufs=5) as gin,
            tc.tile_pool(name="out", bufs=4) as outp,
            tc.tile_pool(name="ps", bufs=8, space="PSUM") as psum,
        ):
            for fi, (g_dram, o_dram, T2l) in enumerate(
                ((gc, sc, T2c), (gs, ss, T2s))
            ):
                # label-agnostic DMA groups over the whole tile stream
                kts = _p1_groups(sum(T2l), ramp=(fi == 0))
                bounds = []
                acc = 0
                for kt in kts:
                    bounds.append((acc, kt))
                    acc += kt
                tile_grp = []  # global tile idx -> (group idx, idx in group)
                for gidx, (g0, kt) in enumerate(bounds):
                    for i in range(kt):
                        tile_grp.append((gidx, i))
                gtiles = [None] * len(kts)
                off = 0
                gi = 0
                for l in range(L):
                    T2 = T2l[l]
                    if T2 == 0:
                        continue
                    ps0 = psum.tile([128, N], mybir.dt.float32, tag="ps")
                    ps1 = psum.tile([128, 128], mybir.dt.float32, tag="ps")
                    for n in range(T2):
                        gidx, k = tile_grp[gi]
                        if k == 0:
                            KT = kts[gidx]
                            t = gin.tile(
                                [128, P1_KT, 2, ROW], mybir.dt.float8e4, tag="g"
                            )
                            src = g_dram[off : off + 128 * KT * 2 * ROW].rearrange(
                                "(p t j c) -> p t j c", p=128, t=KT, j=2, c=ROW
                            )
                            nc.sync.dma_start(t[:, 0:KT], src)
                            off += 128 * KT * 2 * ROW
                            gtiles[gidx] = t
                        t = gtiles[gidx]
                        nc.tensor.matmul(
                            ps0[:], t[:, k, :, 0:128], t[:, k, :, :],
                            start=(n == 0), stop=(n == T2 - 1), perf_mode=DR,
                        )
                        nc.tensor.matmul(
                            ps1[:], t[:, k, :, 128:256], t[:, k, :, 128:256],
                            start=(n == 0), stop=(n == T2 - 1), perf_mode=DR,
                        )
                        gi += 1
                    ob = outp.tile([128, W], mybir.dt.float16, tag="o")
                    nc.vector.tensor_copy(ob[:, 0:N], ps0[:])
                    nc.vector.tensor_copy(ob[:, N:W], ps1[:])
                    # scalar HWDGE ring: keep the sync ring free for inputs
                    nc.scalar.dma_start(o_dram[l], ob[:])
    nc.compile()
    return nc


def _build_phase2(C2s, N):
    """g2: (N, P2) bf16 channel-major gathered content (per-label blocks
    of C2s[l] pixels).
    tq: (128, L, 2, 2, 128) bf16 with tq[k,l,j,i,m] = T_l[i*128+m, j*128+k].
    bi: (128, 2, L) f32 with bi[p,i,l] = b_l[i*128+p].
    oc: (N, P2) bf16 colored output (channel-major, gathered order)."""
    assert N == 256
    L = len(C2s)
    P2 = sum(C2s)
    lab_off = np.concatenate(([0], np.cumsum(C2s))).astype(int)

    nc = bacc.Bacc("TRN2", target_bir_lowering=False, debug=False, num_devices=NCORES)
    g2 = nc.dram_tensor("g2", [N, P2], mybir.dt.bfloat16, kind="ExternalInput")
    tq = nc.dram_tensor("tq", [128, L, 2, 2, 128], mybir.dt.bfloat16, kind="ExternalInput")
    bi = nc.dram_tensor("bi", [128, 2, L], mybir.dt.float32, kind="ExternalInput")
    oc = nc.dram_tensor("oc", [N, P2], mybir.dt.bfloat16, kind="ExternalOutput")

    groups_by_label = _p2_groups(C2s)

    with tile.TileContext(nc) as tc:
        with (
            tc.tile_pool(name="const", bufs=1) as constp,
            tc.tile_pool(name="gin", bufs=8) as gin,
            tc.tile_pool(name="out", bufs=8) as outp,
            tc.tile_pool(name="ps", bufs=4, space="PSUM") as psum,
        ):
            # constants on the scalar ring so the first pixel-block DMA is
            # not queued behind them on the sync ring
            tqt = constp.tile([128, L, 2, 2, 128], mybir.dt.bfloat16)
            nc.scalar.dma_start(tqt[:], tq[:])
            bit = constp.tile([128, 2, L], mybir.dt.float32)
            nc.scalar.dma_start(bit[:], bi[:])

            g2r = g2[:].rearrange("(j k) x -> k j x", j=2)
            ocr2 = oc[:].rearrange("(i k) x -> k i x", i=2)
            for l in range(L):
                base = int(lab_off[l])
                for off, G, subs in groups_by_label[l]:
                    gt = gin.tile([128, 2, 1024], mybir.dt.bfloat16, tag="g")
                    nc.sync.dma_start(
                        gt[:, :, 0:G], g2r[:, :, base + off : base + off + G]
                    )
                    # both i-chunks evict into one tile -> a single output
                    # DMA per group
                    ob = outp.tile([128, 2, 1024], mybir.dt.bfloat16, tag="o")
                    for i in range(2):
                        for so, S in subs:
                            # one PSUM bank per <=512-px sub-block
                            ps = psum.tile([128, 512], mybir.dt.float32, tag="ps")
                            nc.tensor.matmul(
                                ps[:, 0:S], tqt[:, l, 0, i, :],
                                gt[:, 0, so : so + S], start=True, stop=False,
                            )
                            nc.tensor.matmul(
                                ps[:, 0:S], tqt[:, l, 1, i, :],
                                gt[:, 1, so : so + S], start=False, stop=True,
                            )
                            # evictions split across the two elementwise
                            # engines so neither stalls PSUM recycling
                            if i == 0:
                                nc.vector.tensor_scalar_add(
                                    ob[:, 0, so : so + S], ps[:, 0:S],
                                    bit[:, i, l : l + 1],
                                )
                            else:
                                nc.scalar.activation(
                                    ob[:, 1, so : so + S], ps[:, 0:S],
                                    mybir.ActivationFunctionType.Identity,
                                    bias=bit[:, i, l : l + 1],
                                )
                    # gpsimd HWDGE ring: keeps the scalar engine free for
                    # evictions and the sync ring free for inputs
                    nc.gpsimd.dma_start(
                        ocr2[:, :, base + off : base + off + G], ob[:, :, 0:G]
                    )
    nc.compile()
    return nc


def _run(nc, in_maps, label):
    if TRACE:
        import os
        import shutil

        tdir = f"{TRACE_DIR}/{label}"
        shutil.rmtree(tdir, ignore_errors=True)
        os.makedirs(tdir, exist_ok=True)
        res = run_bass_kernel_spmd(
            nc, in_maps, list(range(NCORES)), trace=True, tmpdir=tdir
        )
        LAST_NS[label] = res.exec_time_ns
    else:
        res = run_bass_kernel_spmd(nc, in_maps, list(range(NCORES)))
    return res


def kernel(content_feat, style_feat, content_seg, style_seg, num_labels):
    L = int(num_labels)
    B, N, H, W = content_feat.shape
    M = H * W
    assert B == 1 and N == 256

    c = np.asarray(content_feat, dtype=np.float32).reshape(N, M)
    s = np.asarray(style_feat, dtype=np.float32).reshape(N, M)
    seg_c = np.asarray(content_seg).reshape(M).astype(np.int64)
    seg_s = np.asarray(style_seg).reshape(M).astype(np.int64)

    order_c = np.argsort(seg_c, kind="stable")
    order_s = np.argsort(seg_s, kind="stable")
    counts_c = np.bincount(seg_c, minlength=L)[:L]
    counts_s = np.bincount(seg_s, minlength=L)[:L]

    def split_counts(cnt):
        base = cnt // NCORES
        out = np.tile(base[:, None], (1, NCORES))
        for l in range(L):
            out[l, : cnt[l] % NCORES] += 1
        return out

    cc = split_counts(counts_c)  # (L, NCORES)
    cs = split_counts(counts_s)

    # phase-1 per-label 256-px tile counts; phase-2 per-label 128-px caps
    T2c = [int(_round_up(cc[l].max(), 256)) // 256 for l in range(L)]
    T2s = [int(_round_up(cs[l].max(), 256)) // 256 for l in range(L)]
    C2s = [int(_round_up(cc[l].max(), 128)) for l in range(L)]
    lab_off2 = np.concatenate(([0], np.cumsum(C2s))).astype(int)
    P2 = int(lab_off2[-1])

    # sort the pixel-major views once; everything else slices contiguously
    cT32 = np.ascontiguousarray(c.T)  # (M, N)
    sorted_c = cT32[order_c]  # (M, N) label-sorted pixel-major content
    sorted_s = np.ascontiguousarray(s.T)[order_s]
    lab_pos_c = np.concatenate(([0], np.cumsum(counts_c))).astype(int)
    lab_pos_s = np.concatenate(([0], np.cumsum(counts_s))).astype(int)

    # exact (f64) per-label channel sums on the host; the device computes
    # only the raw second moments
    sum_c_host = np.zeros((L, N), dtype=np.float64)
    sum_s_host = np.zeros((L, N), dtype=np.float64)
    for l in range(L):
        sum_c_host[l] = sorted_c[lab_pos_c[l] : lab_pos_c[l + 1]].sum(
            axis=0, dtype=np.float64
        )
        sum_s_host[l] = sorted_s[lab_pos_s[l] : lab_pos_s[l + 1]].sum(
            axis=0, dtype=np.float64
        )

    sorted_c_f8 = sorted_c.astype(FP8)
    sorted_s_f8 = sorted_s.astype(FP8)

    def build_gather_p1(sx_f8, core_counts, lab_pos, T2l, ramp):
        """Per core, a flat fp8 array: the whole feature's 256-px tile
        stream (per-label zero-padded blocks back to back), chunked into
        label-agnostic _p1_groups DMA groups, each laid out
        (128, KT, 2, ROW); pixel t*256+j*128+p of the stream lands at
        [p, t, j, :] of its group."""
        ntiles = sum(T2l)
        kts = _p1_groups(ntiles, ramp=ramp)
        arrs = []
        for k in range(NCORES):
            tiles = np.zeros((ntiles * 256, ROW), dtype=FP8)
            t0 = 0
            for l in range(len(T2l)):
                T2 = T2l[l]
                if T2 == 0:
                    continue
                m = int(core_counts[l, k])
                off = int(lab_pos[l]) + sum(int(core_counts[l, kk]) for kk in range(k))
                if m:
                    tiles[t0 * 256 : t0 * 256 + m] = sx_f8[off : off + m]
                t0 += T2
            tiles = tiles.reshape(ntiles, 2, 128, ROW)
            out = np.empty(ntiles * 128 * 2 * ROW, dtype=FP8)
            pos = 0
            g0 = 0
            for kt in kts:
                nel = kt * 128 * 2 * ROW
                out[pos : pos + nel] = (
                    tiles[g0 : g0 + kt].transpose(2, 0, 1, 3).reshape(-1)
                )
                pos += nel
                g0 += kt
            arrs.append(out)
        return arrs

    gc_arrs = build_gather_p1(sorted_c_f8, cc, lab_pos_c, T2c, ramp=True)
    gs_arrs = build_gather_p1(sorted_s_f8, cs, lab_pos_s, T2s, ramp=False)
    del sorted_c_f8, sorted_s_f8, sorted_s

    # kick off phase-2 build + a dummy warm-up run in the background so its
    # NEFF compile overlaps phase 1's (wall-clock only; device results of the
    # dummy run are discarded). Falls back to the serial path on any failure.
    p2_box = {}

    def _precompile_p2():
        try:
            nc2 = _build_phase2(C2s, N)
            if PRECOMPILE_WARM:
                z = {
                    "g2": np.zeros((N, P2), dtype=BF16),
                    "tq": np.zeros((128, L, 2, 2, 128), dtype=BF16),
                    "bi": np.zeros((128, 2, L), dtype=np.float32),
                }
                run_bass_kernel_spmd(nc2, [z] * NCORES, list(range(NCORES)))
            p2_box["nc"] = nc2
        except Exception as e:  # pragma: no cover - fallback path
            p2_box["err"] = e

    import threading

    p2_thread = threading.Thread(target=_precompile_p2, daemon=True)
    p2_thread.start()

    nc1p = _build_phase1(T2c, T2s, N)
    if TRACE:
        # keep the traced phase-1 profile free of the background warm-up run
        p2_thread.join()
    res1 = _run(
        nc1p,
        [{"gc": gc_arrs[k], "gs": gs_arrs[k]} for k in range(NCORES)],
        "p1",
    )

    # host: all-reduce moments, finish stats, cholesky, transforms (float64)
    PW = 2 * N - 128
    sc_sum = np.zeros((L, 128, PW), dtype=np.float64)
    ss_sum = np.zeros((L, 128, PW), dtype=np.float64)
    for k in range(NCORES):
        sc_sum += res1.results[k]["sc"]
        ss_sum += res1.results[k]["ss"]

    def unpack(ssum, l):
        Sm = np.empty((N, N), dtype=np.float64)
        Sm[0:128, :] = ssum[l, :, 0:N]
        Sm[128:N, 128:N] = ssum[l, :, N:PW]
        Sm[128:N, 0:128] = Sm[0:128, 128:N].T
        return Sm

    eyeN = np.eye(N, dtype=np.float64)
    T_all = np.zeros((L, N, N), dtype=np.float64)
    b_all = np.zeros((L, N), dtype=np.float64)
    valid = np.zeros(L, dtype=bool)

    try:
        from scipy.linalg import solve_triangular as _st

        def tri_inv(Lm):
            return _st(Lm, eyeN, lower=True)
    except ImportError:

        def tri_inv(Lm):
            return np.linalg.solve(Lm, eyeN)

    for l in range(L):
        ncnt = float(counts_c[l])
        nsnt = float(counts_s[l])
        v = (ncnt > 10) and (nsnt > 10) and (ncnt < 100.0 * nsnt) and (nsnt < 100.0 * ncnt)
        Tl, bl = eyeN, np.zeros(N)
        if v:
            Sc = unpack(sc_sum, l)
            Ss = unpack(ss_sum, l)
            mc = sum_c_host[l] / max(ncnt, 1.0)
            ms = sum_s_host[l] / max(nsnt, 1.0)
            cov_c = (Sc - ncnt * np.outer(mc, mc)) / max(max(ncnt, 1.0) - 1.0, 1.0)
            cov_s = (Ss - nsnt * np.outer(ms, ms)) / max(max(nsnt, 1.0) - 1.0, 1.0)
            try:
                Lc = np.linalg.cholesky(cov_c)
                Ls = np.linalg.cholesky(cov_s)
                Tl = Ls @ tri_inv(Lc)
                bl = ms - Tl @ mc
            except np.linalg.LinAlgError:
                v, Tl, bl = False, eyeN, np.zeros(N)
        T_all[l], b_all[l], valid[l] = Tl, bl, v

    # phase-2 inputs
    tq_np = np.zeros((128, L, 2, 2, 128), dtype=BF16)
    for l in range(L):
        Tl = T_all[l].astype(np.float32)
        for j in range(2):
            for i in range(2):
                tq_np[:, l, j, i, :] = Tl[
                    i * 128 : (i + 1) * 128, j * 128 : (j + 1) * 128
                ].T
    bi_np = np.zeros((128, 2, L), dtype=np.float32)
    for l in range(L):
        for i in range(2):
            bi_np[:, i, l] = b_all[l][i * 128 : (i + 1) * 128]

    g2_arrs = []
    for k in range(NCORES):
        g2 = np.zeros((N, P2), dtype=BF16)
        for l in range(L):
            m = int(cc[l, k])
            if m:
                off = int(lab_pos_c[l]) + sum(int(cc[l, kk]) for kk in range(k))
                dst = int(lab_off2[l])
                g2[:, dst : dst + m] = sorted_c[off : off + m].astype(BF16).T
        g2_arrs.append(g2)

    p2_thread.join()
    nc2p = p2_box.get("nc")
    if nc2p is None:
        nc2p = _build_phase2(C2s, N)
    res2 = _run(
        nc2p,
        [{"g2": g2_arrs[k], "tq": tq_np, "bi": bi_np} for k in range(NCORES)],
        "p2",
    )

    # assemble: gathered order -> sorted order -> original pixel order
    sorted_pm = np.empty((M, N), dtype=np.float32)
    pos = 0
    for l in range(L):
        for k in range(NCORES):
            m = int(cc[l, k])
            if m:
                if valid[l]:
                    dst = int(lab_off2[l])
                    sorted_pm[pos : pos + m] = np.asarray(
                        res2.results[k]["oc"].T[dst : dst + m], dtype=np.float32
                    )
                else:
                    sorted_pm[pos : pos + m] = sorted_c[pos : pos + m]
            pos += m

    # pixels whose label is outside [0, L) are untouched by the reference
    if pos < M:
        sorted_pm[pos:] = sorted_c[pos:]

    final_pm = np.empty((M, N), dtype=np.float32)
    final_pm[order_c] = sorted_pm
    return np.ascontiguousarray(final_pm.T).reshape(B, N, H, W)
